# revision 1
# baseline (speedup 1.0000x reference)
"""BitNet attention block on 8 Trainium2 NeuronCores.

Sharding: sequence-parallel. Each core owns 256 of the 2048 tokens and
computes its tokens' QKV projection, attention (against the full K/V,
obtained via AllGather — v first so it hides behind rope, then k),
RMSNorm and output projection. Weights are pre-transposed/stripped and
cast to fp16 on the host; quantized activations are integers in
[-128, 127], which fp16 represents exactly, so the only precision loss
in the matmuls is the fp16 weight/value rounding (~5e-4 relative).

All per-token scales are folded into PSUM-eviction scalars:
  - dequant scale 1/s (and 1/sqrt(128) for q) on the QKV eviction
  - softmax denominator via a ones-column appended to V (the AV matmul
    accumulates row sums into column 128), applied on the AV eviction
  - RMSNorm rsqrt and second dequant scale on the final WO eviction
    (quant_input and RMSNorm are both row-scale-invariant, so the rsqrt
    never has to touch the full tensor)
Softmax needs no max-subtraction: scores are bounded (|s| < ~10) because
the inputs are absmax-quantized; exp(s - 8) keeps the fp16 exp outputs
comfortably in range (softmax is shift-invariant, so the -8 cancels).
"""

import sys

if '/opt/trn_rl_repo' not in sys.path:
    sys.path.insert(0, '/opt/trn_rl_repo')

import numpy as np
import ml_dtypes

import concourse.bass as bass
import concourse.bacc as bacc
import concourse.tile as tile
from concourse import mybir
from concourse.bass_utils import run_bass_kernel_spmd
from concourse.masks import make_identity

dt = mybir.dt

N_CORES = 8
S = 2048
SL = S // N_CORES            # 256 tokens per core
TCH = SL // 128              # 2 token chunks of 128
H = 2560
NQ, NKV, D = 20, 5, 128
G = NQ // NKV                # 4 query heads per kv head
QKV_N = 3840
KV_W = 2 * NKV * D           # 1280 (k then v)
NOC = 8                      # qkv output column chunks
OC_W = QKV_N // NOC          # 480
WOC = 5                      # wo output column chunks
WOC_W = H // WOC             # 512
NKC = S // 128               # 16 key chunks
MAGIC = 3.0 * 2.0 ** 22      # fp32 round-to-nearest-even forcing constant
INV127 = 1.0 / 127.0
ISQRT_D = 1.0 / float(np.sqrt(128.0))
EPS = 1e-5
THETA = 500000.0


def _build(apply_nw: bool, use_collective: bool = True, lim=None):
    lim = lim or {}
    n_heads = lim.get('heads', NQ)
    n_qoc = lim.get('qkv_oc', NOC)
    n_woc = lim.get('wo_oc', WOC)
    _ORDER = ['xqT', 'qkv', 'rope', 'ag', 'ld', 'attn', 'k2', 'wo']
    _stop = lim.get('stop', 'wo')
    _pad = lim.get('pad', 0)

    def on(stage):
        return _ORDER.index(stage) <= _ORDER.index(_stop)
    nc = bacc.Bacc("TRN2", target_bir_lowering=False, debug=False,
                   num_devices=N_CORES)

    xs_d = nc.dram_tensor("xs", [SL, H], dt.float32, kind="ExternalInput")
    wq_d = nc.dram_tensor("wq", [NOC, 20, 128, OC_W], dt.float16,
                          kind="ExternalInput")
    wo_d = nc.dram_tensor("wow", [WOC, 20, 128, WOC_W], dt.float16,
                          kind="ExternalInput")
    if apply_nw:
        nw_d = nc.dram_tensor("nw", [128, H], dt.float32, kind="ExternalInput")
    cos_d = nc.dram_tensor("cosh", [128, TCH, 64], dt.float32,
                           kind="ExternalInput")
    sin_d = nc.dram_tensor("sinh", [128, TCH, 64], dt.float32,
                           kind="ExternalInput")
    ys_d = nc.dram_tensor("ys", [SL, H], dt.float32, kind="ExternalOutput")

    def _pad_spin(psPool, sbPool, dep_ap_f16):
        # dependency-chained PE spin of known duration (timing variants)
        pt = sbPool.tile([128, 512], dt.float16, name="padsrc", bufs=1)
        nc.vector.tensor_copy(pt, dep_ap_f16)
        pps = psPool.tile([128, 512], dt.float32, name="padps", bufs=1)
        for i in range(_pad):
            nc.tensor.matmul(pps, pt[:, 0:128], pt,
                             start=(i == 0), stop=(i == _pad - 1))

    with tile.TileContext(nc) as tc:
        with (
            tc.tile_pool(name="persist", bufs=1) as pp,
            tc.tile_pool(name="dram", bufs=1, space="DRAM") as dram,
        ):
            ident = pp.tile([128, 128], dt.float16)
            make_identity(nc, ident)
            cos_sb = pp.tile([128, TCH, 64], dt.float32)
            sin_sb = pp.tile([128, TCH, 64], dt.float32)
            nc.sync.dma_start(out=cos_sb, in_=cos_d[:, :, :])
            nc.sync.dma_start(out=sin_sb, in_=sin_d[:, :, :])
            if apply_nw:
                nw_sb = pp.tile([128, H], dt.float32)
                nc.sync.dma_start(out=nw_sb, in_=nw_d[:, :])
            eps_sb = pp.tile([128, 1], dt.float32)
            nc.vector.memset(eps_sb, EPS)
            nbias_sb = pp.tile([128, 1], dt.float32)
            nc.vector.memset(nbias_sb, -8.0)

            m2acc = pp.tile([128, TCH], dt.float32)  # running absmax of ao
            nc.vector.memset(m2acc, 0.0)
            ssqacc = pp.tile([128, TCH], dt.float32)  # running sum(ao^2)
            nc.vector.memset(ssqacc, 0.0)
            rq = pp.tile([128, TCH], dt.float32)    # q dequant scale / sqrt(D)
            rkv = pp.tile([128, TCH], dt.float32)   # kv dequant scale
            qT = pp.tile([128, NQ, SL], dt.float16)
            ao = pp.tile([128, TCH, H], dt.float32)  # normalized attn out

            cc_in = dram.tile([2, 128, 1280], dt.float16)
            cc_out = dram.tile([N_CORES, 2, 128, 1280], dt.float16,
                               addr_space="Shared" if use_collective
                               else "Local")

            # ------------ stage 1+2+3: quant, qkv matmul, rope -----------
            with (
                tc.tile_pool(name="s1", bufs=1) as s1,
                tc.tile_pool(name="wpool", bufs=40) as wp,
                tc.tile_pool(name="psT", bufs=4, space="PSUM") as psT,
                tc.tile_pool(name="psMM", bufs=4, space="PSUM") as psMM,
            ):
                kTl = s1.tile([128, NKV, SL], dt.float16)
                if n_qoc < NOC:
                    pass  # reduced-variant timing builds memset unused bufs

                vb = s1.tile([128, TCH, NKV * D], dt.float16)
                xqT = s1.tile([128, 20, SL], dt.float16)
                qn = s1.tile([128, TCH, H], dt.float32)
                kvn = s1.tile([128, TCH, KV_W], dt.float32)
                if n_qoc < NOC:
                    nc.vector.memset(qn, 0.0)
                    nc.vector.memset(kvn, 0.0)

                for tch in range(TCH):
                    xt = s1.tile([128, H], dt.float32, tag="xt", bufs=2)
                    nc.sync.dma_start(out=xt,
                                      in_=xs_d[tch * 128:(tch + 1) * 128, :])
                    m = s1.tile([128, 1], dt.float32, tag="m", bufs=2)
                    nc.vector.tensor_reduce(out=m, in_=xt,
                                            op=mybir.AluOpType.max,
                                            axis=mybir.AxisListType.X,
                                            apply_absolute_value=True)
                    rm = s1.tile([128, 1], dt.float32, tag="rm", bufs=2)
                    nc.vector.reciprocal(rm, m)
                    rs = s1.tile([128, 1], dt.float32, tag="rs", bufs=2)
                    nc.vector.tensor_scalar_mul(rs, rm, 127.0)
                    nc.vector.tensor_scalar_mul(rkv[:, tch:tch + 1], m, INV127)
                    nc.vector.tensor_scalar_mul(rq[:, tch:tch + 1], m,
                                                INV127 * ISQRT_D)
                    xm = s1.tile([128, H], dt.float32, tag="xm", bufs=2)
                    nc.vector.tensor_scalar(out=xm, in0=xt, scalar1=rs,
                                            scalar2=None,
                                            op0=mybir.AluOpType.mult)
                    xq = s1.tile([128, H], dt.float16, tag="xq", bufs=2)
                    nc.vector.tensor_scalar(out=xq, in0=xm, scalar1=MAGIC,
                                            scalar2=MAGIC,
                                            op0=mybir.AluOpType.add,
                                            op1=mybir.AluOpType.subtract)
                    for ic in range(20):
                        tp = psT.tile([128, 128], dt.float16, tag="tp")
                        nc.tensor.transpose(tp, xq[:, ic * 128:(ic + 1) * 128],
                                            ident)
                        nc.vector.tensor_copy(
                            xqT[:, ic, tch * 128:(tch + 1) * 128], tp)

                # qkv matmul over streamed weight strips; kv columns first so
                # the AllGather can launch under the q-column matmuls
                _psb = 3 if (_pad and _stop == 'qkv') else 4
                _oc_order = [5, 6, 7, 0, 1, 2, 3, 4][:n_qoc] if n_qoc == NOC \
                    else list(range(n_qoc))
                _kv_ocs = [oc for oc in _oc_order if oc >= 5]
                _q_ocs = [oc for oc in _oc_order if oc < 5]

                def _qkv_chunk(oc):
                    pss = [psMM.tile([128, OC_W], dt.float32, tag="ps",
                                     name=f"ps_{oc}_{t}", bufs=_psb)
                           for t in range(TCH)]
                    for ic2 in range(10):
                        wt = wp.tile([128, 2, OC_W], dt.float16, tag="wt",
                                     bufs=28)
                        weng = nc.sync if ic2 % 2 == 0 else nc.gpsimd
                        weng.dma_start(
                            out=wt,
                            in_=wq_d[oc, 2 * ic2:2 * ic2 + 2, :, :]
                            .rearrange("two p n -> p two n"))
                        for u in range(2):
                            ic = 2 * ic2 + u
                            for tch in range(TCH):
                                nc.tensor.matmul(
                                    pss[tch],
                                    xqT[:, ic, tch * 128:(tch + 1) * 128],
                                    wt[:, u, :],
                                    start=(ic == 0), stop=(ic == 19))
                    lo = oc * OC_W
                    hi = lo + OC_W
                    for tch in range(TCH):
                        # split the eviction at the q|kv boundary (col 2560)
                        if hi <= H:
                            nc.vector.tensor_scalar(
                                out=qn[:, tch, lo:hi], in0=pss[tch],
                                scalar1=rq[:, tch:tch + 1], scalar2=None,
                                op0=mybir.AluOpType.mult)
                        elif lo >= H:
                            nc.vector.tensor_scalar(
                                out=kvn[:, tch, lo - H:hi - H], in0=pss[tch],
                                scalar1=rkv[:, tch:tch + 1], scalar2=None,
                                op0=mybir.AluOpType.mult)
                        else:
                            cut = H - lo
                            nc.vector.tensor_scalar(
                                out=qn[:, tch, lo:H], in0=pss[tch][:, 0:cut],
                                scalar1=rq[:, tch:tch + 1], scalar2=None,
                                op0=mybir.AluOpType.mult)
                            nc.vector.tensor_scalar(
                                out=kvn[:, tch, 0:hi - H],
                                in0=pss[tch][:, cut:OC_W],
                                scalar1=rkv[:, tch:tch + 1], scalar2=None,
                                op0=mybir.AluOpType.mult)

                if on('qkv'):
                    for oc in _kv_ocs:
                        _qkv_chunk(oc)

                if _pad and _stop == 'xqT':
                    _pad_spin(psMM, s1, xqT[:, 0:2, :].rearrange("p a b -> p (a b)")[:, 0:512])
                if _pad and _stop == 'qkv':
                    kvc = s1.tile([128, 512], dt.float16, name="kvc", bufs=1)
                    nc.vector.tensor_copy(kvc, kvn[:, TCH - 1, 0:512])
                    _pad_spin(psMM, s1, kvc)
                # v cast + rope-k now, so the AllGather hides under the
                # q-column matmuls
                if on('rope'):
                    for tch in range(TCH):
                        nc.vector.tensor_copy(vb[:, tch, :],
                                              kvn[:, tch, NKV * D:KV_W])
                        nc.sync.dma_start(
                            out=cc_in[1, :, tch * 640:(tch + 1) * 640],
                            in_=vb[:, tch, :])
                for tch in (range(TCH) if on('rope') else []):
                    c_sl = cos_sb[:, tch, :]
                    s_sl = sin_sb[:, tch, :]
                    for (src, nheads, dstT) in (
                        (kvn[:, tch, 0:NKV * D], NKV, kTl),
                    ):
                        cb = c_sl[:, None, :].broadcast_to((128, nheads, 64))
                        sb = s_sl[:, None, :].broadcast_to((128, nheads, 64))
                        v3 = src.rearrange("p (h x) -> p h x", x=128)
                        h1 = v3[:, :, 0:64]
                        h2 = v3[:, :, 64:128]
                        t1 = s1.tile([128, nheads, 64], dt.float32, tag="t1",
                                     bufs=1)
                        t2 = s1.tile([128, nheads, 64], dt.float32, tag="t2",
                                     bufs=1)
                        rr = s1.tile([128, nheads, 128], dt.float16,
                                     tag="rr", bufs=1)
                        nc.vector.tensor_tensor(out=t1, in0=h1, in1=cb,
                                                op=mybir.AluOpType.mult)
                        nc.vector.tensor_tensor(out=t2, in0=h2, in1=sb,
                                                op=mybir.AluOpType.mult)
                        nc.vector.tensor_tensor(out=rr[:, :, 0:64], in0=t1,
                                                in1=t2,
                                                op=mybir.AluOpType.subtract)
                        nc.vector.tensor_tensor(out=t1, in0=h2, in1=cb,
                                                op=mybir.AluOpType.mult)
                        nc.vector.tensor_tensor(out=t2, in0=h1, in1=sb,
                                                op=mybir.AluOpType.mult)
                        nc.vector.tensor_tensor(out=rr[:, :, 64:128], in0=t1,
                                                in1=t2, op=mybir.AluOpType.add)
                        for h in range(nheads):
                            tp = psT.tile([128, 128], dt.float16, tag="tp")
                            nc.tensor.transpose(tp, rr[:, h, :], ident)
                            nc.vector.tensor_copy(
                                dstT[:, h, tch * 128:(tch + 1) * 128], tp)

                if on('ag'):
                    nc.sync.dma_start(out=cc_in[0],
                                      in_=kTl.rearrange("p h s -> p (h s)"))
                if on('ag') and use_collective:
                    nc.gpsimd.collective_compute(
                        "AllGather", mybir.AluOpType.bypass,
                        replica_groups=[list(range(N_CORES))],
                        ins=[cc_in[:, :, :].opt()],
                        outs=[cc_out[:, :, :, :].opt()],
                    )
                elif on('ag'):
                    for c in range(N_CORES):
                        nc.sync.dma_start(out=cc_out[c], in_=cc_in[:, :, :])

                if on('qkv'):
                    for oc in _q_ocs:
                        _qkv_chunk(oc)

                # rope-q + transposes (overlaps the AllGather)
                for tch in (range(TCH) if on('rope') else []):
                    c_sl = cos_sb[:, tch, :]
                    s_sl = sin_sb[:, tch, :]
                    for (src, nheads, dstT) in (
                        (qn[:, tch, :], NQ, qT),
                    ):
                        cb = c_sl[:, None, :].broadcast_to((128, nheads, 64))
                        sb = s_sl[:, None, :].broadcast_to((128, nheads, 64))
                        v3 = src.rearrange("p (h x) -> p h x", x=128)
                        h1 = v3[:, :, 0:64]
                        h2 = v3[:, :, 64:128]
                        t1 = s1.tile([128, nheads, 64], dt.float32, tag="t1",
                                     bufs=1)
                        t2 = s1.tile([128, nheads, 64], dt.float32, tag="t2",
                                     bufs=1)
                        rr = s1.tile([128, nheads, 128], dt.float16,
                                     tag="rr", bufs=1)
                        nc.vector.tensor_tensor(out=t1, in0=h1, in1=cb,
                                                op=mybir.AluOpType.mult)
                        nc.vector.tensor_tensor(out=t2, in0=h2, in1=sb,
                                                op=mybir.AluOpType.mult)
                        nc.vector.tensor_tensor(out=rr[:, :, 0:64], in0=t1,
                                                in1=t2,
                                                op=mybir.AluOpType.subtract)
                        nc.vector.tensor_tensor(out=t1, in0=h2, in1=cb,
                                                op=mybir.AluOpType.mult)
                        nc.vector.tensor_tensor(out=t2, in0=h1, in1=sb,
                                                op=mybir.AluOpType.mult)
                        nc.vector.tensor_tensor(out=rr[:, :, 64:128], in0=t1,
                                                in1=t2, op=mybir.AluOpType.add)
                        for h in range(nheads):
                            tp = psT.tile([128, 128], dt.float16, tag="tp")
                            nc.tensor.transpose(tp, rr[:, h, :], ident)
                            nc.vector.tensor_copy(
                                dstT[:, h, tch * 128:(tch + 1) * 128], tp)

            # ------------ stage 4+5: attention; stage 6+7: norm + wo -----
            with (
                tc.tile_pool(name="att", bufs=1) as at,
                tc.tile_pool(name="s3", bufs=1) as s3,
                tc.tile_pool(name="wpool2", bufs=40) as wp2,
            ):
                with (
                    tc.tile_pool(name="psS", bufs=3, space="PSUM") as psS,
                    tc.tile_pool(name="psA", bufs=4, space="PSUM") as psA,
                ):
                    KT = at.tile([128, NKV, S], dt.float16)
                    if on('ld'):
                        for g in range(NKV):
                            eng = nc.sync if g % 2 == 0 else nc.gpsimd
                            eng.dma_start(
                                out=KT[:, g, :].rearrange(
                                    "p (c s) -> p c s", c=N_CORES),
                                in_=cc_out[:, 0, :, g * SL:(g + 1) * SL]
                                .rearrange("c p s -> p c s"))
                    # V with a ones column per kv head: [128, 16, 5, 129]
                    Va = at.tile([128, NKC, NKV, D + 1], dt.float16)
                    if on('ld'):
                        nc.gpsimd.memset(Va, 1.0)
                    for c in (range(N_CORES) if on('ld') else []):
                        for tch in range(TCH):
                            j = c * TCH + tch
                            base = tch * 640
                            eng = nc.sync if j % 2 == 0 else nc.gpsimd
                            eng.dma_start(
                                out=Va[:, j, :, 0:D],
                                in_=cc_out[c, 1, :, base:base + 640]
                                .rearrange("p (g d) -> p g d", g=NKV))

                    if _pad and _stop == 'ld':
                        _pad_spin(psS, at, Va[:, NKC - 1, :, :].rearrange(
                            "p a b -> p (a b)")[:, 0:512])
                    for hp in (range(n_heads // 2) if on('attn') else []):
                        h0 = 2 * hp
                        g = h0 // G
                        ex_t = at.tile([128, NKC, 2, SL], dt.float16,
                                       tag="ex", bufs=3)
                        for j in range(NKC):
                            sp = psS.tile([128, 2 * SL], dt.float32, tag="sp")
                            nc.tensor.matmul(
                                sp, KT[:, g, j * 128:(j + 1) * 128],
                                qT[:, h0:h0 + 2, :].rearrange(
                                    "p a b -> p (a b)"),
                                start=True, stop=True)
                            nc.scalar.activation(
                                ex_t[:, j, :, :].rearrange("p a b -> p (a b)"),
                                sp, mybir.ActivationFunctionType.Exp,
                                bias=nbias_sb)
                        for hh in range(2):
                            for tch in range(TCH):
                                ap_ps = psA.tile([128, D + 1], dt.float32,
                                                 tag="ap")
                                for j in range(NKC):
                                    nc.tensor.matmul(
                                        ap_ps,
                                        ex_t[:, j, hh,
                                             tch * 128:(tch + 1) * 128],
                                        Va[:, j, g, :],
                                        start=(j == 0), stop=(j == NKC - 1))
                                rr = at.tile([128, 1], dt.float32, tag="rsum",
                                             bufs=2)
                                nc.vector.reciprocal(rr, ap_ps[:, D:D + 1])
                                nc.vector.tensor_scalar(
                                    out=ao[:, tch,
                                           (h0 + hh) * D:(h0 + hh + 1) * D],
                                    in0=ap_ps[:, 0:D], scalar1=rr,
                                    scalar2=None, op0=mybir.AluOpType.mult)
                                pm = at.tile([128, 1], dt.float32, tag="pm",
                                             bufs=2)
                                nc.vector.tensor_reduce(
                                    out=pm,
                                    in_=ao[:, tch,
                                           (h0 + hh) * D:(h0 + hh + 1) * D],
                                    op=mybir.AluOpType.max,
                                    axis=mybir.AxisListType.X,
                                    apply_absolute_value=True)
                                nc.vector.tensor_tensor(
                                    out=m2acc[:, tch:tch + 1],
                                    in0=m2acc[:, tch:tch + 1], in1=pm,
                                    op=mybir.AluOpType.max)
                                sc2 = at.tile([128, D], dt.float32,
                                              tag="sc2", bufs=2)
                                nc.vector.tensor_tensor(
                                    out=sc2,
                                    in0=ao[:, tch,
                                           (h0 + hh) * D:(h0 + hh + 1) * D],
                                    in1=ao[:, tch,
                                           (h0 + hh) * D:(h0 + hh + 1) * D],
                                    op=mybir.AluOpType.mult)
                                psq = at.tile([128, 1], dt.float32,
                                              tag="psq", bufs=2)
                                nc.vector.tensor_reduce(
                                    out=psq, in_=sc2, op=mybir.AluOpType.add,
                                    axis=mybir.AxisListType.X)
                                nc.vector.tensor_tensor(
                                    out=ssqacc[:, tch:tch + 1],
                                    in0=ssqacc[:, tch:tch + 1], in1=psq,
                                    op=mybir.AluOpType.add)

                with (
                    tc.tile_pool(name="psT2", bufs=4, space="PSUM") as psT2,
                    tc.tile_pool(name="psY", bufs=4, space="PSUM") as psY,
                ):
                    if _pad and _stop == 'attn':
                        aoc = s3.tile([128, 512], dt.float16, name="aoc", bufs=1)
                        nc.vector.tensor_copy(aoc, ao[:, TCH - 1, H - 512:H])
                        _pad_spin(psY, s3, aoc)
                    k2T = s3.tile([128, 20, SL], dt.float16)
                    ry = s3.tile([128, TCH], dt.float32)
                    for tch in (range(TCH) if on('k2') else []):
                        if apply_nw:
                            u = s3.tile([128, H], dt.float32, tag="u", bufs=1)
                            nc.vector.tensor_tensor(out=u, in0=ao[:, tch, :],
                                                    in1=nw_sb,
                                                    op=mybir.AluOpType.mult)
                        else:
                            u = ao[:, tch, :]
                        # RMS statistic uses the raw attention output; the
                        # sum of squares was accumulated during attention
                        scr = s3.tile([128, H], dt.float32, tag="scr", bufs=1)
                        if apply_nw or n_heads < NQ:
                            ssq = s3.tile([128, 1], dt.float32, tag="ssq",
                                          bufs=2)
                            nc.scalar.activation(
                                scr, ao[:, tch, :],
                                mybir.ActivationFunctionType.Square,
                                accum_out=ssq)
                        else:
                            ssq = ssqacc[:, tch:tch + 1]
                        sq = s3.tile([128, 1], dt.float32, tag="sq", bufs=2)
                        nc.scalar.activation(
                            sq, ssq, mybir.ActivationFunctionType.Sqrt,
                            bias=eps_sb, scale=1.0 / H)
                        rsv = s3.tile([128, 1], dt.float32, tag="rsv", bufs=2)
                        nc.vector.reciprocal(rsv, sq)
                        if apply_nw or n_heads < NQ:
                            m2 = s3.tile([128, 1], dt.float32, tag="m2",
                                         bufs=2)
                            nc.vector.tensor_reduce(
                                out=m2, in_=u, op=mybir.AluOpType.max,
                                axis=mybir.AxisListType.X,
                                apply_absolute_value=True)
                        else:
                            m2 = m2acc[:, tch:tch + 1]
                        rm2 = s3.tile([128, 1], dt.float32, tag="rm2", bufs=2)
                        nc.vector.reciprocal(rm2, m2)
                        rs2 = s3.tile([128, 1], dt.float32, tag="rs2", bufs=2)
                        nc.vector.tensor_scalar_mul(rs2, rm2, 127.0)
                        is2 = s3.tile([128, 1], dt.float32, tag="is2", bufs=2)
                        nc.vector.tensor_scalar_mul(is2, m2, INV127)
                        nc.vector.tensor_tensor(out=ry[:, tch:tch + 1],
                                                in0=rsv, in1=is2,
                                                op=mybir.AluOpType.mult)
                        # reuse scr for the scaled values
                        nc.vector.tensor_scalar(out=scr, in0=u, scalar1=rs2,
                                                scalar2=None,
                                                op0=mybir.AluOpType.mult)
                        k2 = s3.tile([128, H], dt.float16, tag="k2", bufs=1)
                        nc.vector.tensor_scalar(out=k2, in0=scr, scalar1=MAGIC,
                                                scalar2=MAGIC,
                                                op0=mybir.AluOpType.add,
                                                op1=mybir.AluOpType.subtract)
                        for ic in range(20):
                            tp = psT2.tile([128, 128], dt.float16, tag="tp2")
                            nc.tensor.transpose(
                                tp, k2[:, ic * 128:(ic + 1) * 128], ident)
                            nc.vector.tensor_copy(
                                k2T[:, ic, tch * 128:(tch + 1) * 128], tp)

                    for oc in (range(n_woc) if on('wo') else []):
                        pss = [psY.tile([128, WOC_W], dt.float32, tag="py",
                                        name=f"py_{oc}_{t}")
                               for t in range(TCH)]
                        for ic2 in range(10):
                            wt2 = wp2.tile([128, 2, WOC_W], dt.float16,
                                           tag="wt2", bufs=20)
                            weng = nc.sync if ic2 % 2 == 0 else nc.gpsimd
                            weng.dma_start(
                                out=wt2,
                                in_=wo_d[oc, 2 * ic2:2 * ic2 + 2, :, :]
                                .rearrange("two p n -> p two n"))
                            for u in range(2):
                                ic = 2 * ic2 + u
                                for tch in range(TCH):
                                    nc.tensor.matmul(
                                        pss[tch],
                                        k2T[:, ic, tch * 128:(tch + 1) * 128],
                                        wt2[:, u, :],
                                        start=(ic == 0), stop=(ic == 19))
                        for tch in range(TCH):
                            yt = s3.tile([128, WOC_W], dt.float32, tag="yt",
                                         bufs=3)
                            nc.vector.tensor_scalar(
                                out=yt, in0=pss[tch],
                                scalar1=ry[:, tch:tch + 1],
                                scalar2=None, op0=mybir.AluOpType.mult)
                            nc.sync.dma_start(
                                out=ys_d[tch * 128:(tch + 1) * 128,
                                         oc * WOC_W:(oc + 1) * WOC_W],
                                in_=yt)

    nc.compile()
    return nc


_CACHE = {}


def _prep_host(x, wqkv, wo, norm_w):
    x = np.asarray(x, np.float32)
    wqkv = np.asarray(wqkv, np.float32)
    wo = np.asarray(wo, np.float32)
    norm_w = np.asarray(norm_w, np.float32)

    xs = np.ascontiguousarray(x.reshape(S, H))
    wqkvT = np.ascontiguousarray(wqkv.T)           # [H, QKV_N]
    wq_strips = np.ascontiguousarray(
        wqkvT.reshape(20, 128, NOC, OC_W).transpose(2, 0, 1, 3)).astype(np.float16)
    woT = np.ascontiguousarray(wo.T)               # [H, H]
    wo_strips = np.ascontiguousarray(
        woT.reshape(20, 128, WOC, WOC_W).transpose(2, 0, 1, 3)).astype(np.float16)
    nw_b = np.ascontiguousarray(np.broadcast_to(norm_w[None, :], (128, H)))

    inv_freq = (1.0 / (np.float32(THETA) **
                       (np.arange(0, D, 2, dtype=np.float32) / np.float32(D))))
    t = np.arange(S, dtype=np.float32)
    freqs = np.outer(t, inv_freq).astype(np.float32)   # [S, 64]
    cos = np.cos(freqs).astype(np.float32)
    sin = np.sin(freqs).astype(np.float32)

    in_maps = []
    for c in range(N_CORES):
        sl = slice(c * SL, (c + 1) * SL)
        cos_c = np.ascontiguousarray(
            cos[sl].reshape(TCH, 128, 64).transpose(1, 0, 2))
        sin_c = np.ascontiguousarray(
            sin[sl].reshape(TCH, 128, 64).transpose(1, 0, 2))
        in_maps.append({
            "xs": np.ascontiguousarray(xs[sl]),
            "wq": wq_strips,
            "wow": wo_strips,
            "nw": nw_b,
            "cosh": cos_c,
            "sinh": sin_c,
        })
    return in_maps


def kernel(x, wqkv, wo, norm_w):
    apply_nw = not np.allclose(np.asarray(norm_w, np.float32), 1.0)
    key = ('nc', apply_nw)
    if key not in _CACHE:
        _CACHE[key] = _build(apply_nw)
    nc = _CACHE[key]
    in_maps = _prep_host(x, wqkv, wo, norm_w)
    if not apply_nw:
        for m in in_maps:
            m.pop("nw")
    res = run_bass_kernel_spmd(nc, in_maps, list(range(N_CORES)))
    out = np.concatenate([res.results[c]["ys"] for c in range(N_CORES)],
                         axis=0)
    return out.reshape(1, S, H).astype(np.float32)

